# revision 29
# baseline (speedup 1.0000x reference)
"""Trainium2 Bass kernel for local-window sparse attention.

Problem: B=32, N=1024 tokens (16x64 grid), C=768, 12 heads x 64 dims,
local 7x11 window additive mask, qkv proj + attention + out proj.

Strategy: data-parallel over batch across 8 NeuronCores (4 batches per
core).  The end-to-end wall clock is dominated by the ~45 MB/s axon
host<->device tunnel, so the host-side driver is built around
minimizing wire bytes and per-call overhead:

  - the compiled PJRT executable is cached across calls with the C++
    fast-dispatch path (the stock run_bass_kernel_spmd path re-traces +
    re-compiles every call)
  - weights / mask / output-staging buffers are uploaded once and kept
    device-resident (re-uploaded only if the weight arrays change)
  - x is sent token-major as per-token int8 with an fp32 scale packed
    per row -> [B, N, C+4] uint8 (25 MB instead of 100 MB fp32); it is
    quantized per-core-chunk on the host so the CPU quantization of
    chunk c+1 overlaps the wire transfer of chunk c
  - the output is quantized on-device to per-token int8 + fp32 scale
    -> [B, N, C+4] uint8 (25 MB back instead of 100 MB); per-shard
    host dequant overlaps the remaining shard transfers

Per-core kernel:
  - x tiles arrive token-major int8; dequant = int8->bf16 copy +
    per-partition scale multiply, then 128x128 TensorE transposes
    (via identity) build the feature-major xT
  - qT/kT = W_chunk.T @ xT  (feature-major, heads packed 2-per-128)
  - v produced token-major with an extra all-ones column per head
    (so the PV matmul also produces the softmax denominator as row 64)
  - scoresT[j] = kT_h.T @ qT_h  (keys on partitions, queries on free)
    exp via ScalarE (scale=1/8 folded in), 0/1 band mask mul on VectorE
  - avT = v_ext.T @ attnT accumulated over key tiles, normalized by the
    denominator row, written feature-major
  - out = avT.T @ W_proj + b_proj, then per-token abs-max int8 quant

Only key tiles intersecting the local band are computed, and within
each (key-tile, query-chunk) pair the scores matmul / exp / mask / PV
matmul are restricted to the in-band query column subrange.
"""

import numpy as np
import ml_dtypes

import concourse.mybir as mybir
import concourse.tile as tile
from concourse import bacc
from concourse import bass2jax
from concourse.bass import ds

# ---- problem constants (hardcoded; kernel.py must be self-contained) ----
B, N, C = 32, 1024, 768
HEADS, D = 12, 64
H, W, HK, WK = 16, 64, 7, 11
NCORES = 8
BL = B // NCORES  # batches per core
KT = C // 128     # 6 contraction tiles over embed dim
NT = N // 128     # 8 token tiles
BF16 = mybir.dt.bfloat16
F32 = mybir.dt.float32
U8 = mybir.dt.uint8
I8 = mybir.dt.int8

OROW = C + 4      # int8 row + packed fp32 scale (shared by x and out)

ROWS_PER_KTILE = 128 // W  # 2 grid rows per 128-token tile
RH = HK // 2               # 3: half-window in grid rows
MB = 512                   # stored mask band width per key tile


def _mask_base(j):
    """First query column stored in the banded mask for key tile j."""
    return min(max(64 * (2 * j - 3), 0), N - MB)


def _band_tiles(qc, qchunk=512):
    """Key tiles j intersecting the band for query chunk qc (512 queries)."""
    qr0, qr1 = (qchunk // W) * qc, (qchunk // W) * (qc + 1) - 1  # grid rows
    jlo = max(0, (qr0 - RH) // ROWS_PER_KTILE)
    jhi = min(NT - 1, (qr1 + RH) // ROWS_PER_KTILE)
    return list(range(jlo, jhi + 1))


def _qsub(j, qc, qchunk=512):
    """In-band query column subrange [lo, hi) within chunk qc for key tile j.

    Key tile j covers grid rows [2j, 2j+1]; in-band query grid rows are
    [2j - RH, 2j + 1 + RH] clipped to the chunk. Returns offsets relative
    to chunk start, multiples of W=64.
    """
    rows_per_chunk = qchunk // W
    qr_lo = max(ROWS_PER_KTILE * j - RH, rows_per_chunk * qc)
    qr_hi = min(ROWS_PER_KTILE * j + (ROWS_PER_KTILE - 1) + RH,
                rows_per_chunk * (qc + 1) - 1)
    lo = qr_lo * W - qchunk * qc
    hi = (qr_hi + 1) * W - qchunk * qc
    return lo, hi


def build_kernel(nbatch=BL, subrange=True):
    nc = bacc.Bacc(None, target_bir_lowering=False)
    xq_d = nc.declare_dram_parameter("xq", [nbatch, N, OROW], U8, isOutput=False)
    wqkv_d = nc.declare_dram_parameter("wqkv", [C, 3 * C], BF16, isOutput=False)
    wproj_d = nc.declare_dram_parameter("wproj", [C, C], BF16, isOutput=False)
    bproj_d = nc.declare_dram_parameter("bproj", [1, C], BF16, isOutput=False)
    maskT_d = nc.declare_dram_parameter("maskT", [N, MB], BF16, isOutput=False)
    eye_d = nc.declare_dram_parameter("eye", [128, 128], BF16, isOutput=False)
    out_d = nc.declare_dram_parameter("out", [nbatch, N, OROW], U8, isOutput=True)

    with tile.TileContext(nc) as tc:
        with (
            tc.tile_pool(name="weights", bufs=1) as wpool,
            tc.tile_pool(name="xq", bufs=4) as xqpool,
            tc.tile_pool(name="xtm", bufs=5) as xtmpool,
            tc.tile_pool(name="xt", bufs=2) as xpool,
            tc.tile_pool(name="acts", bufs=2) as qkpool,
            tc.tile_pool(name="acts1", bufs=2) as avpool,
            tc.tile_pool(name="attn", bufs=4) as apool,
            tc.tile_pool(name="outs", bufs=2) as opool,
            tc.tile_pool(name="small", bufs=2) as spool,
            tc.tile_pool(name="gemm_ps", bufs=2, space="PSUM") as gemm_ps,
            tc.tile_pool(name="sc_ps", bufs=2, space="PSUM") as scpool,
            tc.tile_pool(name="pv_ps", bufs=2, space="PSUM") as pvpool,
        ):
            # ---- persistent weights in SBUF (eye/wqkv/x(0) are DMA'd
            # first, below, so the first transposes + QKV matmuls
            # aren't stuck behind the mask/wproj burst in the HWDGE
            # FIFO) ----
            wqkv_s = wpool.tile([128, KT, 3 * C], BF16)
            wproj_s = wpool.tile([128, KT, C], BF16)
            maskT_s = wpool.tile([128, NT, MB], BF16)
            bproj_s = wpool.tile([1, C], BF16)
            eye_s = wpool.tile([128, 128], BF16)

            ones_s = wpool.tile([1, 128], BF16)
            nc.vector.memset(ones_s[:], 1.0)
            zero65_s = wpool.tile([1, 65], BF16)
            nc.vector.memset(zero65_s[:], 0.0)

            acts = {}

            def load_x(b):
                # x arrives token-major int8 + per-token fp32 scale;
                # dequant on Vector/Scalar, transpose 128x128 blocks on
                # TensorE (via identity) to build feature-major xT.
                xT_s = xpool.tile([128, KT, N], BF16, tag="xT", name=f"xT{b}")
                for tth in range(2):
                    xtms = []
                    for tq in range(4):
                        tt = 4 * tth + tq
                        xq_t = xqpool.tile([128, OROW], U8, tag="xq")
                        nc.sync.dma_start(xq_t[:], xq_d[b, ds(128 * tt, 128), :])
                        xtm = xtmpool.tile([128, C], BF16, tag="xtm")
                        nc.vector.tensor_copy(xtm[:], xq_t[:, 0:C].bitcast(I8))
                        nc.scalar.mul(xtm[:], xtm[:],
                                      xq_t[:, C:OROW].bitcast(F32))
                        xtms.append(xtm)
                    for j in range(KT):
                        tp = gemm_ps.tile([128, 512], BF16, tag="gemm",
                                          name="tp")
                        for tq in range(4):
                            nc.tensor.transpose(
                                tp[:, ds(128 * tq, 128)],
                                xtms[tq][:, ds(128 * j, 128)], eye_s[:])
                        nc.vector.tensor_copy(
                            xT_s[:, j, ds(512 * tth, 512)], tp[:])
                acts[b] = {"xT": xT_s}

            def qkv_groups(b):
                xT_s = acts[b]["xT"]
                qT_s = qkpool.tile([128, KT, N], BF16, tag="qT", name=f"qT{b}")
                kT_s = qkpool.tile([128, KT, N], BF16, tag="kT", name=f"kT{b}")
                vext_s = qkpool.tile([128, NT, HEADS, D + 1], BF16, tag="vext",
                                     name=f"vext{b}")
                acts[b].update(qT=qT_s, kT=kT_s, vext=vext_s)
                groups = [lambda: nc.vector.memset(vext_s[:, :, :, D:D + 1], 1.0)]

                def qk_group(ft, qc2):
                    dest = qT_s if ft < KT else kT_s
                    p = ft % KT
                    ps = gemm_ps.tile([128, 512], F32, tag="gemm", name="psqk")
                    for j in range(KT):
                        nc.tensor.matmul(
                            ps[:],
                            wqkv_s[:, j, ds(ft * 128, 128)],
                            xT_s[:, j, ds(qc2 * 512, 512)],
                            start=(j == 0), stop=(j == KT - 1),
                        )
                    nc.vector.tensor_copy(dest[:, p, ds(qc2 * 512, 512)], ps[:])

                def v_group(tt, nck):
                    ps = gemm_ps.tile([128, 512], F32, tag="gemm", name="psv")
                    for j in range(KT):
                        nc.tensor.matmul(
                            ps[:, 0:384],
                            xT_s[:, j, ds(tt * 128, 128)],
                            wqkv_s[:, j, ds(2 * C + nck * 384, 384)],
                            start=(j == 0), stop=(j == KT - 1),
                        )
                    nc.vector.tensor_copy(
                        vext_s[:, tt, ds(6 * nck, 6), 0:D],
                        ps[:, 0:384].rearrange("p (h d) -> p h d", d=D),
                    )

                for ft in range(2 * KT):
                    for qc2 in range(2):
                        groups.append(lambda ft=ft, qc2=qc2: qk_group(ft, qc2))
                for tt in range(NT):
                    for nck in range(2):
                        groups.append(lambda tt=tt, nck=nck: v_group(tt, nck))
                return groups

            def attn_part1(b, hp, qc):
                qT_s, kT_s = acts[b]["qT"], acts[b]["kT"]
                vext_s = acts[b]["vext"]
                js = _band_tiles(qc)
                pv = [pvpool.tile([65, 512], F32, tag="pv", name=f"pv{_h}")
                      for _h in range(2)]
                for half in range(2):
                    nc.tensor.matmul(
                        pv[half][:], zero65_s[:], maskT_s[0:1, 0, 0:512],
                        start=True, stop=False, skip_group_check=True,
                    )
                for ji, j in enumerate(js):
                    lo, hi = _qsub(j, qc) if subrange else (0, 512)
                    w = hi - lo
                    sc = scpool.tile([128, 2, 512], F32, tag="sc")
                    et = apool.tile([128, 2, 512], BF16, tag="et")
                    for half in range(2):
                        nc.tensor.matmul(
                            sc[ds(0, 128), half, ds(0, w)],
                            kT_s[ds(64 * half, 64), hp, ds(128 * j, 128)],
                            qT_s[ds(64 * half, 64), hp, ds(512 * qc + lo, w)],
                            start=True, stop=True,
                        )
                    nc.scalar.activation(
                        et[:, :, ds(0, w)], sc[:, :, ds(0, w)],
                        mybir.ActivationFunctionType.Exp, scale=0.125,
                    )
                    nc.vector.tensor_mul(
                        et[:, :, ds(0, w)],
                        et[:, :, ds(0, w)],
                        maskT_s[:, j, ds(512 * qc + lo - _mask_base(j), w)]
                        .rearrange("p (a n) -> p a n", a=1)
                        .broadcast_to((128, 2, w)),
                    )
                    for half in range(2):
                        nc.tensor.matmul(
                            pv[half][ds(0, 65), ds(lo, w)],
                            vext_s[:, j, 2 * hp + half, 0:65],
                            et[:, half, ds(0, w)],
                            start=False,
                            stop=(j == js[-1]),
                            skip_group_check=True,
                        )
                return pv

            def attn_part2(b, hp, qc, pv):
                avT_s = acts[b]["avT"]
                avu = apool.tile([128, 512], BF16, tag="avu")
                rb = gemm_ps.tile([128, 512], F32, tag="gemm", name="rb")
                for half in range(2):
                    nc.vector.tensor_copy(avu[ds(64 * half, 64), :],
                                          pv[half][0:64, :])
                    rec = spool.tile([1, 512], F32, tag="rec")
                    nc.vector.reciprocal(rec[:], pv[half][64:65, :])
                    recb = spool.tile([1, 512], BF16, tag="recb")
                    nc.vector.tensor_copy(recb[:], rec[:])
                    nc.tensor.matmul(rb[ds(64 * half, 64), :],
                                     ones_s[:, 0:64], recb[:],
                                     start=True, stop=True)
                nc.vector.tensor_mul(
                    avT_s[:, hp, ds(qc * 512, 512)], avu[:], rb[:],
                )

            def proj_groups(b, tts):
                avT_s = acts[b]["avT"]
                sdall_s = acts[b]["sdall"]

                def proj_tile(tt):
                    oat = opool.tile([128, C], F32, tag="oat")
                    for nck in range(2):
                        ps = gemm_ps.tile([128, 512], F32, tag="gemm", name="psp")
                        nc.tensor.matmul(
                            ps[:, 0:384], ones_s[:, 0:128],
                            bproj_s[:, ds(nck * 384, 384)],
                            start=True, stop=False,
                        )
                        for j in range(KT):
                            nc.tensor.matmul(
                                ps[:, 0:384],
                                avT_s[:, j, ds(tt * 128, 128)],
                                wproj_s[:, j, ds(nck * 384, 384)],
                                start=False, stop=(j == KT - 1),
                            )
                        nc.vector.tensor_copy(oat[:, ds(nck * 384, 384)],
                                              ps[:, 0:384])
                    # per-token int8 quantization: q = round(out * 127/absmax)
                    m = spool.tile([128, 1], F32, tag="m")
                    nc.vector.tensor_reduce(
                        m[:], oat[:], axis=mybir.AxisListType.X,
                        op=mybir.AluOpType.max, apply_absolute_value=True)
                    r = spool.tile([128, 1], F32, tag="r")
                    nc.vector.reciprocal(r[:], m[:])
                    r127 = spool.tile([128, 1], F32, tag="r127")
                    nc.scalar.mul(r127[:], r[:], 127.0)
                    nc.scalar.mul(sdall_s[:, ds(tt, 1)], m[:], 1.0 / 127.0)
                    q8 = opool.tile([128, C], I8, tag="q8")
                    nc.scalar.activation(
                        q8[:], oat[:], mybir.ActivationFunctionType.Copy,
                        scale=r127[:])
                    nc.sync.dma_start(out_d[b, ds(tt * 128, 128), 0:C],
                                      q8[:].bitcast(U8))
                    if tt == NT - 1:
                        nc.sync.dma_start(
                            out_d[b, :, C:OROW]
                            .rearrange("(t p) w -> p t w", p=128),
                            sdall_s[:].bitcast(U8)
                            .rearrange("p (t w) -> p t w", w=4),
                        )

                return [lambda tt=tt: proj_tile(tt) for tt in tts]

            # software pipeline: interleave QKV(b+1) / proj(b-1) groups
            # between attention(b) iterations (emission order only; all
            # per-op code is identical to the serial version)
            from collections import deque
            import math
            nc.sync.dma_start(eye_s[:], eye_d[:])
            for j in range(KT):
                nc.sync.dma_start(wqkv_s[:, j, :], wqkv_d[ds(128 * j, 128), :])
            load_x(0)
            nc.sync.dma_start(maskT_s[:], maskT_d[:].rearrange("(j p) n -> p j n", p=128))
            nc.sync.dma_start(wproj_s[:], wproj_d[:].rearrange("(j p) f -> p j f", p=128))
            nc.sync.dma_start(bproj_s[:], bproj_d[:])
            for g in qkv_groups(0):
                g()
            pending = deque()
            for b in range(nbatch):
                acts[b]["avT"] = avpool.tile([128, KT, N], BF16, tag="avT",
                                             name=f"avT{b}")
                acts[b]["sdall"] = avpool.tile([128, NT], F32, tag="sdall",
                                               name=f"sdall{b}")
                if b + 1 < nbatch:
                    load_x(b + 1)
                    pending.extend(qkv_groups(b + 1))
                iters = [(hp, qc) for qc in range(2) for hp in range(KT)]

                def fill(k):
                    for _ in range(min(k, len(pending))):
                        pending.popleft()()

                for i, (hp, qc) in enumerate(iters):
                    quota = min(math.ceil(len(pending) / (len(iters) - i)), 4)
                    pv = attn_part1(b, hp, qc)
                    fill(4)
                    attn_part2(b, hp, qc, pv)
                    fill(quota - 4)
                    if i == KT - 1:
                        # qc=0 done: proj tiles over tokens [0, 512) are ready
                        pending.extend(proj_groups(b, range(NT // 2)))
                pending.extend(proj_groups(b, range(NT // 2, NT)))
                if b > 0 and b - 1 in acts:
                    del acts[b - 1]
            while pending:
                pending.popleft()()

    nc.compile()
    return nc


def _local_mask_T():
    """Binary (1=in-window) local mask, transposed: maskT[m, n]."""
    m = np.ones((N, H + HK - 1, W + WK - 1), dtype=np.float32)
    for h in range(H):
        for w in range(W):
            m[h * W + w, h:h + HK, w:w + WK] = 0.0
    mp = m[:, HK // 2:H + HK // 2, WK // 2:W + WK // 2].reshape(N, N)
    binm = (mp < 1.0).astype(np.float32)
    return np.ascontiguousarray(binm.T)


_CACHE = {}


def _host_fns():
    """jax-CPU jits for quantize / dequantize (multithreaded)."""
    import jax
    import jax.numpy as jnp

    cpu = jax.devices("cpu")[0]

    def _quant_x(x):  # [BL, N, C] f32 -> [BL, N, C+4] u8 (token-major)
        s = jnp.max(jnp.abs(x), axis=2, keepdims=True)
        s = jnp.maximum(s, 1e-30)
        # no clip needed: |x * 127/s| <= 127 by construction
        q = jnp.round(x * (127.0 / s))
        qu = jax.lax.bitcast_convert_type(q.astype(jnp.int8), jnp.uint8)
        sd = (s * (1.0 / 127.0)).astype(jnp.float32)
        su = jax.lax.bitcast_convert_type(sd, jnp.uint8).reshape(BL, N, 4)
        return jnp.concatenate([qu, su], axis=2)

    scratch = {}

    def _dequant_chunk(buf, res):  # [BL, N, C+4] u8 -> res [BL, N, C] f32
        q = buf[:, :, :C].view(np.int8)
        s = np.ascontiguousarray(buf[:, :, C:OROW]).view(np.float32)
        np.multiply(q, s, out=res, dtype=np.float32)

    def _dequant_out(out_arr):  # sharded [B, N, C+4] u8 -> [B, N, C] f32
        # per-shard fetch + dequant: dequant of shard i overlaps the wire
        # transfer of shard i+1. Output buffer reused across calls (the
        # contents are deterministic per input so reuse is safe).
        if "out" not in scratch:
            scratch["out"] = np.empty((B, N, C), np.float32)
        res = scratch["out"]
        out_arr.copy_to_host_async()
        for shd in out_arr.addressable_shards:
            _dequant_chunk(np.asarray(shd.data), res[shd.index[0]])
        return res

    return cpu, jax.jit(_quant_x), _dequant_out


def _state():
    if "st" in _CACHE:
        return _CACHE["st"]
    import jax
    from jax.sharding import Mesh, PartitionSpec, NamedSharding
    try:
        from jax.experimental.shard_map import shard_map
    except ImportError:
        from jax import shard_map

    nc = build_kernel(BL)
    bass2jax.install_neuronx_cc_hook()

    partition_name = (nc.partition_id_tensor.name
                      if nc.partition_id_tensor else None)
    in_names, out_names, out_avals = [], [], []
    for alloc in nc.m.functions[0].allocations:
        if not isinstance(alloc, mybir.MemoryLocationSet):
            continue
        name = alloc.memorylocations[0].name
        if alloc.kind == "ExternalInput":
            if name != partition_name:
                in_names.append(name)
        elif alloc.kind == "ExternalOutput":
            out_names.append(name)
            out_avals.append(jax.core.ShapedArray(
                tuple(alloc.tensor_shape), mybir.dt.np(alloc.dtype)))
    n_params = len(in_names)
    all_in_names = list(in_names) + out_names
    if partition_name is not None:
        all_in_names.append(partition_name)

    def _body(*args):
        operands = list(args)
        if partition_name is not None:
            operands.append(bass2jax.partition_id_tensor())
        outs = bass2jax._bass_exec_p.bind(
            *operands, out_avals=tuple(out_avals),
            in_names=tuple(all_in_names), out_names=tuple(out_names),
            lowering_input_output_aliases=(),
            sim_require_finite=True, sim_require_nnan=True, nc=nc)
        return tuple(outs)

    devices = jax.devices()[:NCORES]
    mesh = Mesh(np.asarray(devices), ("core",))
    n_out = len(out_names)
    fn = jax.jit(
        shard_map(_body, mesh=mesh,
                  in_specs=(PartitionSpec("core"),) * (n_params + n_out),
                  out_specs=(PartitionSpec("core"),) * n_out,
                  check_rep=False),
        keep_unused=True)
    sh = NamedSharding(mesh, PartitionSpec("core"))

    # AOT compile once with the C++ fast-dispatch path (no bass_effect)
    gshape = {}
    for alloc in nc.m.functions[0].allocations:
        if not isinstance(alloc, mybir.MemoryLocationSet):
            continue
        if alloc.kind in ("ExternalInput", "ExternalOutput"):
            nm = alloc.memorylocations[0].name
            shp = tuple(alloc.tensor_shape)
            gshape[nm] = jax.ShapeDtypeStruct(
                (NCORES * shp[0],) + shp[1:], mybir.dt.np(alloc.dtype),
                sharding=sh)
    sds = [gshape[nm] for nm in all_in_names if nm != partition_name]
    try:
        compiled = bass2jax.fast_dispatch_compile(
            lambda: fn.lower(*sds).compile())
    except Exception:
        compiled = fn

    cpu, quant_x, dequant_out = _host_fns()

    # out staging buffers: uploaded once, never donated, so the NEFF's
    # result buffers are freshly allocated each call (kernel writes
    # every output element; no zero-init needed)
    zeros = jax.device_put(
        np.zeros((NCORES * BL, N, OROW), np.uint8), sh)

    st = dict(nc=nc, fn=compiled, sh=sh, cpu=cpu, quant_x=quant_x,
              dequant_out=dequant_out, in_names=in_names, zeros=zeros,
              devices=devices, weights=None, dev_w=None)
    _CACHE["st"] = st
    return st


def _upload_weights(st, W_qkv, W_proj, b_proj):
    import jax

    def same(a, b):
        return a is b or np.array_equal(a, b)

    w = st["weights"]
    if (w is not None and same(w[0], W_qkv) and same(w[1], W_proj)
            and same(w[2], b_proj)):
        return
    mfull = _local_mask_T()
    maskT = np.concatenate(
        [mfull[128 * j:128 * (j + 1), _mask_base(j):_mask_base(j) + MB]
         for j in range(NT)], axis=0).astype(ml_dtypes.bfloat16)
    wqkv = W_qkv.astype(ml_dtypes.bfloat16)
    wproj = W_proj.astype(ml_dtypes.bfloat16)
    bproj = np.ascontiguousarray(b_proj.reshape(1, C)).astype(
        ml_dtypes.bfloat16)
    eye = np.eye(128, dtype=np.float32).astype(ml_dtypes.bfloat16)
    dev = {
        "wqkv": np.concatenate([wqkv] * NCORES, axis=0),
        "wproj": np.concatenate([wproj] * NCORES, axis=0),
        "bproj": np.concatenate([bproj] * NCORES, axis=0),
        "maskT": np.concatenate([maskT] * NCORES, axis=0),
        "eye": np.concatenate([eye] * NCORES, axis=0),
    }
    st["dev_w"] = {k: jax.device_put(v, st["sh"]) for k, v in dev.items()}
    st["weights"] = (W_qkv.copy(), W_proj.copy(), b_proj.copy())


def kernel(x, W_qkv, W_proj, b_proj):
    import jax

    x = np.asarray(x, dtype=np.float32)
    W_qkv = np.asarray(W_qkv, dtype=np.float32)
    W_proj = np.asarray(W_proj, dtype=np.float32)
    b_proj = np.asarray(b_proj, dtype=np.float32)

    st = _state()
    _upload_weights(st, W_qkv, W_proj, b_proj)

    # per-core chunk quantize + shard upload: the wire transfer of shard
    # c overlaps the CPU quantization of shard c+1
    shards = []
    for c in range(NCORES):
        with jax.default_device(st["cpu"]):
            qc = st["quant_x"](x[c * BL:(c + 1) * BL])
            qc.block_until_ready()
        shards.append(jax.device_put(qc, st["devices"][c]))
    xd = jax.make_array_from_single_device_arrays(
        (B, N, OROW), st["sh"], shards)

    args = []
    for nm in st["in_names"]:
        args.append(xd if nm == "xq" else st["dev_w"][nm])
    args.append(st["zeros"])
    outs = st["fn"](*args)
    return st["dequant_out"](outs[0])


# revision 30
# speedup vs baseline: 1.0533x; 1.0533x over previous
"""Trainium2 Bass kernel for local-window sparse attention.

Problem: B=32, N=1024 tokens (16x64 grid), C=768, 12 heads x 64 dims,
local 7x11 window additive mask, qkv proj + attention + out proj.

Strategy: data-parallel over batch across 8 NeuronCores (4 batches per
core).  The end-to-end wall clock is dominated by the ~45 MB/s axon
host<->device tunnel, so the host-side driver is built around
minimizing wire bytes and per-call overhead:

  - the compiled PJRT executable is cached across calls with the C++
    fast-dispatch path (the stock run_bass_kernel_spmd path re-traces +
    re-compiles every call)
  - weights / mask / output-staging buffers are uploaded once and kept
    device-resident (re-uploaded only if the weight arrays change)
  - x is sent token-major as per-token int8 with an fp32 scale packed
    per row -> [B, N, C+4] uint8 (25 MB instead of 100 MB fp32); it is
    quantized per-core-chunk on the host so the CPU quantization of
    chunk c+1 overlaps the wire transfer of chunk c
  - the output is quantized on-device to per-token int8 + fp32 scale
    -> [B, N, C+4] uint8 (25 MB back instead of 100 MB); per-shard
    host dequant overlaps the remaining shard transfers

Per-core kernel:
  - x tiles arrive token-major int8; dequant = int8->bf16 copy +
    per-partition scale multiply, then 128x128 TensorE transposes
    (via identity) build the feature-major xT
  - qT/kT = W_chunk.T @ xT  (feature-major, heads packed 2-per-128)
  - v produced token-major with an extra all-ones column per head
    (so the PV matmul also produces the softmax denominator as row 64)
  - scoresT[j] = kT_h.T @ qT_h  (keys on partitions, queries on free)
    exp via ScalarE (scale=1/8 folded in), 0/1 band mask mul on VectorE
  - avT = v_ext.T @ attnT accumulated over key tiles, normalized by the
    denominator row, written feature-major
  - out = avT.T @ W_proj + b_proj, then per-token abs-max int8 quant

Only key tiles intersecting the local band are computed, and within
each (key-tile, query-chunk) pair the scores matmul / exp / mask / PV
matmul are restricted to the in-band query column subrange.
"""

import numpy as np
import ml_dtypes

import concourse.mybir as mybir
import concourse.tile as tile
from concourse import bacc
from concourse import bass2jax
from concourse.bass import ds

# ---- problem constants (hardcoded; kernel.py must be self-contained) ----
B, N, C = 32, 1024, 768
HEADS, D = 12, 64
H, W, HK, WK = 16, 64, 7, 11
NCORES = 8
BL = B // NCORES  # batches per core
KT = C // 128     # 6 contraction tiles over embed dim
NT = N // 128     # 8 token tiles
BF16 = mybir.dt.bfloat16
F32 = mybir.dt.float32
U8 = mybir.dt.uint8
I8 = mybir.dt.int8

OROW = C + 4      # int8 row + packed fp32 scale (shared by x and out)

ROWS_PER_KTILE = 128 // W  # 2 grid rows per 128-token tile
RH = HK // 2               # 3: half-window in grid rows
MB = 512                   # stored mask band width per key tile


def _mask_base(j):
    """First query column stored in the banded mask for key tile j."""
    return min(max(64 * (2 * j - 3), 0), N - MB)


def _band_tiles(qc, qchunk=512):
    """Key tiles j intersecting the band for query chunk qc (512 queries)."""
    qr0, qr1 = (qchunk // W) * qc, (qchunk // W) * (qc + 1) - 1  # grid rows
    jlo = max(0, (qr0 - RH) // ROWS_PER_KTILE)
    jhi = min(NT - 1, (qr1 + RH) // ROWS_PER_KTILE)
    return list(range(jlo, jhi + 1))


def _qsub(j, qc, qchunk=512):
    """In-band query column subrange [lo, hi) within chunk qc for key tile j.

    Key tile j covers grid rows [2j, 2j+1]; in-band query grid rows are
    [2j - RH, 2j + 1 + RH] clipped to the chunk. Returns offsets relative
    to chunk start, multiples of W=64.
    """
    rows_per_chunk = qchunk // W
    qr_lo = max(ROWS_PER_KTILE * j - RH, rows_per_chunk * qc)
    qr_hi = min(ROWS_PER_KTILE * j + (ROWS_PER_KTILE - 1) + RH,
                rows_per_chunk * (qc + 1) - 1)
    lo = qr_lo * W - qchunk * qc
    hi = (qr_hi + 1) * W - qchunk * qc
    return lo, hi


def build_kernel(nbatch=BL, subrange=True):
    nc = bacc.Bacc(None, target_bir_lowering=False)
    xq_d = nc.declare_dram_parameter("xq", [nbatch, N, OROW], U8, isOutput=False)
    wqkv_d = nc.declare_dram_parameter("wqkv", [C, 3 * C], BF16, isOutput=False)
    wproj_d = nc.declare_dram_parameter("wproj", [C, C], BF16, isOutput=False)
    bproj_d = nc.declare_dram_parameter("bproj", [1, C], BF16, isOutput=False)
    maskT_d = nc.declare_dram_parameter("maskT", [N, MB], BF16, isOutput=False)
    eye_d = nc.declare_dram_parameter("eye", [128, 128], BF16, isOutput=False)
    out_d = nc.declare_dram_parameter("out", [nbatch, N, OROW], U8, isOutput=True)

    with tile.TileContext(nc) as tc:
        with (
            tc.tile_pool(name="weights", bufs=1) as wpool,
            tc.tile_pool(name="xq", bufs=4) as xqpool,
            tc.tile_pool(name="xtm", bufs=5) as xtmpool,
            tc.tile_pool(name="xt", bufs=2) as xpool,
            tc.tile_pool(name="acts", bufs=2) as qkpool,
            tc.tile_pool(name="acts1", bufs=2) as avpool,
            tc.tile_pool(name="attn", bufs=4) as apool,
            tc.tile_pool(name="outs", bufs=2) as opool,
            tc.tile_pool(name="small", bufs=2) as spool,
            tc.tile_pool(name="gemm_ps", bufs=2, space="PSUM") as gemm_ps,
            tc.tile_pool(name="sc_ps", bufs=2, space="PSUM") as scpool,
            tc.tile_pool(name="pv_ps", bufs=2, space="PSUM") as pvpool,
        ):
            # ---- persistent weights in SBUF (eye/wqkv/x(0) are DMA'd
            # first, below, so the first transposes + QKV matmuls
            # aren't stuck behind the mask/wproj burst in the HWDGE
            # FIFO) ----
            wqkv_s = wpool.tile([128, KT, 3 * C], BF16)
            wproj_s = wpool.tile([128, KT, C], BF16)
            maskT_s = wpool.tile([128, NT, MB], BF16)
            bproj_s = wpool.tile([1, C], BF16)
            eye_s = wpool.tile([128, 128], BF16)

            ones_s = wpool.tile([1, 128], BF16)
            nc.vector.memset(ones_s[:], 1.0)
            zero65_s = wpool.tile([1, 65], BF16)
            nc.vector.memset(zero65_s[:], 0.0)

            acts = {}

            def load_x(b):
                # x arrives token-major int8 + per-token fp32 scale;
                # dequant on Vector/Scalar, transpose 128x128 blocks on
                # TensorE (via identity) to build feature-major xT.
                xT_s = xpool.tile([128, KT, N], BF16, tag="xT", name=f"xT{b}")
                for tth in range(2):
                    xtms = []
                    for tq in range(4):
                        tt = 4 * tth + tq
                        xq_t = xqpool.tile([128, OROW], U8, tag="xq")
                        nc.sync.dma_start(xq_t[:], xq_d[b, ds(128 * tt, 128), :])
                        xtm = xtmpool.tile([128, C], BF16, tag="xtm")
                        nc.vector.tensor_copy(xtm[:], xq_t[:, 0:C].bitcast(I8))
                        nc.scalar.mul(xtm[:], xtm[:],
                                      xq_t[:, C:OROW].bitcast(F32))
                        xtms.append(xtm)
                    for j in range(KT):
                        tp = gemm_ps.tile([128, 512], BF16, tag="gemm",
                                          name="tp")
                        for tq in range(4):
                            nc.tensor.transpose(
                                tp[:, ds(128 * tq, 128)],
                                xtms[tq][:, ds(128 * j, 128)], eye_s[:])
                        nc.vector.tensor_copy(
                            xT_s[:, j, ds(512 * tth, 512)], tp[:])
                acts[b] = {"xT": xT_s}

            def qkv_groups(b):
                xT_s = acts[b]["xT"]
                qT_s = qkpool.tile([128, KT, N], BF16, tag="qT", name=f"qT{b}")
                kT_s = qkpool.tile([128, KT, N], BF16, tag="kT", name=f"kT{b}")
                vext_s = qkpool.tile([128, NT, HEADS, D + 1], BF16, tag="vext",
                                     name=f"vext{b}")
                acts[b].update(qT=qT_s, kT=kT_s, vext=vext_s)
                groups = [lambda: nc.vector.memset(vext_s[:, :, :, D:D + 1], 1.0)]

                def qk_group(ft, qc2):
                    dest = qT_s if ft < KT else kT_s
                    p = ft % KT
                    ps = gemm_ps.tile([128, 512], F32, tag="gemm", name="psqk")
                    for j in range(KT):
                        nc.tensor.matmul(
                            ps[:],
                            wqkv_s[:, j, ds(ft * 128, 128)],
                            xT_s[:, j, ds(qc2 * 512, 512)],
                            start=(j == 0), stop=(j == KT - 1),
                        )
                    nc.vector.tensor_copy(dest[:, p, ds(qc2 * 512, 512)], ps[:])

                def v_group(tt, nck):
                    ps = gemm_ps.tile([128, 512], F32, tag="gemm", name="psv")
                    for j in range(KT):
                        nc.tensor.matmul(
                            ps[:, 0:384],
                            xT_s[:, j, ds(tt * 128, 128)],
                            wqkv_s[:, j, ds(2 * C + nck * 384, 384)],
                            start=(j == 0), stop=(j == KT - 1),
                        )
                    nc.vector.tensor_copy(
                        vext_s[:, tt, ds(6 * nck, 6), 0:D],
                        ps[:, 0:384].rearrange("p (h d) -> p h d", d=D),
                    )

                for ft in range(2 * KT):
                    for qc2 in range(2):
                        groups.append(lambda ft=ft, qc2=qc2: qk_group(ft, qc2))
                for tt in range(NT):
                    for nck in range(2):
                        groups.append(lambda tt=tt, nck=nck: v_group(tt, nck))
                return groups

            def attn_part1(b, hp, qc):
                qT_s, kT_s = acts[b]["qT"], acts[b]["kT"]
                vext_s = acts[b]["vext"]
                js = _band_tiles(qc)
                pv = [pvpool.tile([65, 512], F32, tag="pv", name=f"pv{_h}")
                      for _h in range(2)]
                for half in range(2):
                    nc.tensor.matmul(
                        pv[half][:], zero65_s[:], maskT_s[0:1, 0, 0:512],
                        start=True, stop=False, skip_group_check=True,
                    )
                for ji, j in enumerate(js):
                    lo, hi = _qsub(j, qc) if subrange else (0, 512)
                    w = hi - lo
                    sc = scpool.tile([128, 2, 512], F32, tag="sc")
                    et = apool.tile([128, 2, 512], BF16, tag="et")
                    for half in range(2):
                        nc.tensor.matmul(
                            sc[ds(0, 128), half, ds(0, w)],
                            kT_s[ds(64 * half, 64), hp, ds(128 * j, 128)],
                            qT_s[ds(64 * half, 64), hp, ds(512 * qc + lo, w)],
                            start=True, stop=True,
                        )
                    nc.scalar.activation(
                        et[:, :, ds(0, w)], sc[:, :, ds(0, w)],
                        mybir.ActivationFunctionType.Exp, scale=0.125,
                    )
                    nc.vector.tensor_mul(
                        et[:, :, ds(0, w)],
                        et[:, :, ds(0, w)],
                        maskT_s[:, j, ds(512 * qc + lo - _mask_base(j), w)]
                        .rearrange("p (a n) -> p a n", a=1)
                        .broadcast_to((128, 2, w)),
                    )
                    for half in range(2):
                        nc.tensor.matmul(
                            pv[half][ds(0, 65), ds(lo, w)],
                            vext_s[:, j, 2 * hp + half, 0:65],
                            et[:, half, ds(0, w)],
                            start=False,
                            stop=(j == js[-1]),
                            skip_group_check=True,
                        )
                return pv

            def attn_part2(b, hp, qc, pv):
                avT_s = acts[b]["avT"]
                avu = apool.tile([128, 512], BF16, tag="avu")
                rb = gemm_ps.tile([128, 512], F32, tag="gemm", name="rb")
                for half in range(2):
                    nc.vector.tensor_copy(avu[ds(64 * half, 64), :],
                                          pv[half][0:64, :])
                    rec = spool.tile([1, 512], F32, tag="rec")
                    nc.vector.reciprocal(rec[:], pv[half][64:65, :])
                    recb = spool.tile([1, 512], BF16, tag="recb")
                    nc.vector.tensor_copy(recb[:], rec[:])
                    nc.tensor.matmul(rb[ds(64 * half, 64), :],
                                     ones_s[:, 0:64], recb[:],
                                     start=True, stop=True)
                nc.vector.tensor_mul(
                    avT_s[:, hp, ds(qc * 512, 512)], avu[:], rb[:],
                )

            def proj_groups(b, tts):
                avT_s = acts[b]["avT"]
                sdall_s = acts[b]["sdall"]

                def proj_tile(tt):
                    oat = opool.tile([128, C], F32, tag="oat")
                    for nck in range(2):
                        ps = gemm_ps.tile([128, 512], F32, tag="gemm", name="psp")
                        nc.tensor.matmul(
                            ps[:, 0:384], ones_s[:, 0:128],
                            bproj_s[:, ds(nck * 384, 384)],
                            start=True, stop=False,
                        )
                        for j in range(KT):
                            nc.tensor.matmul(
                                ps[:, 0:384],
                                avT_s[:, j, ds(tt * 128, 128)],
                                wproj_s[:, j, ds(nck * 384, 384)],
                                start=False, stop=(j == KT - 1),
                            )
                        nc.vector.tensor_copy(oat[:, ds(nck * 384, 384)],
                                              ps[:, 0:384])
                    # per-token int8 quantization: q = round(out * 127/absmax)
                    m = spool.tile([128, 1], F32, tag="m")
                    nc.vector.tensor_reduce(
                        m[:], oat[:], axis=mybir.AxisListType.X,
                        op=mybir.AluOpType.max, apply_absolute_value=True)
                    r = spool.tile([128, 1], F32, tag="r")
                    nc.vector.reciprocal(r[:], m[:])
                    r127 = spool.tile([128, 1], F32, tag="r127")
                    nc.scalar.mul(r127[:], r[:], 127.0)
                    nc.scalar.mul(sdall_s[:, ds(tt, 1)], m[:], 1.0 / 127.0)
                    q8 = opool.tile([128, C], I8, tag="q8")
                    nc.scalar.activation(
                        q8[:], oat[:], mybir.ActivationFunctionType.Copy,
                        scale=r127[:])
                    nc.sync.dma_start(out_d[b, ds(tt * 128, 128), 0:C],
                                      q8[:].bitcast(U8))
                    if tt == NT - 1:
                        nc.sync.dma_start(
                            out_d[b, :, C:OROW]
                            .rearrange("(t p) w -> p t w", p=128),
                            sdall_s[:].bitcast(U8)
                            .rearrange("p (t w) -> p t w", w=4),
                        )

                return [lambda tt=tt: proj_tile(tt) for tt in tts]

            # software pipeline: interleave QKV(b+1) / proj(b-1) groups
            # between attention(b) iterations (emission order only; all
            # per-op code is identical to the serial version)
            from collections import deque
            import math
            nc.sync.dma_start(eye_s[:], eye_d[:])
            for j in range(KT):
                nc.sync.dma_start(wqkv_s[:, j, :], wqkv_d[ds(128 * j, 128), :])
            load_x(0)
            nc.sync.dma_start(maskT_s[:], maskT_d[:].rearrange("(j p) n -> p j n", p=128))
            nc.sync.dma_start(wproj_s[:], wproj_d[:].rearrange("(j p) f -> p j f", p=128))
            nc.sync.dma_start(bproj_s[:], bproj_d[:])
            for g in qkv_groups(0):
                g()
            pending = deque()
            for b in range(nbatch):
                acts[b]["avT"] = avpool.tile([128, KT, N], BF16, tag="avT",
                                             name=f"avT{b}")
                acts[b]["sdall"] = avpool.tile([128, NT], F32, tag="sdall",
                                               name=f"sdall{b}")
                if b + 1 < nbatch:
                    load_x(b + 1)
                    pending.extend(qkv_groups(b + 1))
                iters = [(hp, qc) for qc in range(2) for hp in range(KT)]

                def fill(k):
                    for _ in range(min(k, len(pending))):
                        pending.popleft()()

                for i, (hp, qc) in enumerate(iters):
                    quota = min(math.ceil(len(pending) / (len(iters) - i)), 4)
                    pv = attn_part1(b, hp, qc)
                    fill(4)
                    attn_part2(b, hp, qc, pv)
                    fill(quota - 4)
                    if i == KT - 1:
                        # qc=0 done: proj tiles over tokens [0, 512) are ready
                        pending.extend(proj_groups(b, range(NT // 2)))
                pending.extend(proj_groups(b, range(NT // 2, NT)))
                if b > 0 and b - 1 in acts:
                    del acts[b - 1]
            while pending:
                pending.popleft()()

    nc.compile()
    return nc


def _local_mask_T():
    """Binary (1=in-window) local mask, transposed: maskT[m, n]."""
    m = np.ones((N, H + HK - 1, W + WK - 1), dtype=np.float32)
    for h in range(H):
        for w in range(W):
            m[h * W + w, h:h + HK, w:w + WK] = 0.0
    mp = m[:, HK // 2:H + HK // 2, WK // 2:W + WK // 2].reshape(N, N)
    binm = (mp < 1.0).astype(np.float32)
    return np.ascontiguousarray(binm.T)


_CACHE = {}


def _host_fns():
    """jax-CPU jits for quantize / dequantize (multithreaded)."""
    import jax
    import jax.numpy as jnp

    cpu = jax.devices("cpu")[0]

    def _quant_x(x):  # [BL, N, C] f32 -> [BL, N, C+4] u8 (token-major)
        s = jnp.max(jnp.abs(x), axis=2, keepdims=True)
        s = jnp.maximum(s, 1e-30)
        # no clip needed: |x * 127/s| <= 127 by construction
        q = jnp.round(x * (127.0 / s))
        qu = jax.lax.bitcast_convert_type(q.astype(jnp.int8), jnp.uint8)
        sd = (s * (1.0 / 127.0)).astype(jnp.float32)
        su = jax.lax.bitcast_convert_type(sd, jnp.uint8).reshape(BL, N, 4)
        return jnp.concatenate([qu, su], axis=2)

    scratch = {}

    def _dequant_chunk(buf, res):  # [BL, N, C+4] u8 -> res [BL, N, C] f32
        q = buf[:, :, :C].view(np.int8)
        s = np.ascontiguousarray(buf[:, :, C:OROW]).view(np.float32)
        np.multiply(q, s, out=res, dtype=np.float32)

    def _dequant_out(out_arr):  # sharded [B, N, C+4] u8 -> [B, N, C] f32
        # per-shard fetch + dequant: dequant of shard i overlaps the wire
        # transfer of shard i+1. Output buffer reused across calls (the
        # contents are deterministic per input so reuse is safe).
        if "out" not in scratch:
            scratch["out"] = np.empty((B, N, C), np.float32)
        res = scratch["out"]
        out_arr.copy_to_host_async()
        for shd in out_arr.addressable_shards:
            _dequant_chunk(np.asarray(shd.data), res[shd.index[0]])
        return res

    return cpu, jax.jit(_quant_x), _dequant_out


def _state():
    if "st" in _CACHE:
        return _CACHE["st"]
    import jax
    from jax.sharding import Mesh, PartitionSpec, NamedSharding
    try:
        from jax.experimental.shard_map import shard_map
    except ImportError:
        from jax import shard_map

    nc = build_kernel(BL)
    bass2jax.install_neuronx_cc_hook()

    partition_name = (nc.partition_id_tensor.name
                      if nc.partition_id_tensor else None)
    in_names, out_names, out_avals = [], [], []
    for alloc in nc.m.functions[0].allocations:
        if not isinstance(alloc, mybir.MemoryLocationSet):
            continue
        name = alloc.memorylocations[0].name
        if alloc.kind == "ExternalInput":
            if name != partition_name:
                in_names.append(name)
        elif alloc.kind == "ExternalOutput":
            out_names.append(name)
            out_avals.append(jax.core.ShapedArray(
                tuple(alloc.tensor_shape), mybir.dt.np(alloc.dtype)))
    n_params = len(in_names)
    all_in_names = list(in_names) + out_names
    if partition_name is not None:
        all_in_names.append(partition_name)

    def _body(*args):
        operands = list(args)
        if partition_name is not None:
            operands.append(bass2jax.partition_id_tensor())
        outs = bass2jax._bass_exec_p.bind(
            *operands, out_avals=tuple(out_avals),
            in_names=tuple(all_in_names), out_names=tuple(out_names),
            lowering_input_output_aliases=(),
            sim_require_finite=True, sim_require_nnan=True, nc=nc)
        return tuple(outs)

    devices = jax.devices()[:NCORES]
    mesh = Mesh(np.asarray(devices), ("core",))
    n_out = len(out_names)
    fn = jax.jit(
        shard_map(_body, mesh=mesh,
                  in_specs=(PartitionSpec("core"),) * (n_params + n_out),
                  out_specs=(PartitionSpec("core"),) * n_out,
                  check_rep=False),
        keep_unused=True)
    sh = NamedSharding(mesh, PartitionSpec("core"))

    # AOT compile once with the C++ fast-dispatch path (no bass_effect)
    gshape = {}
    for alloc in nc.m.functions[0].allocations:
        if not isinstance(alloc, mybir.MemoryLocationSet):
            continue
        if alloc.kind in ("ExternalInput", "ExternalOutput"):
            nm = alloc.memorylocations[0].name
            shp = tuple(alloc.tensor_shape)
            gshape[nm] = jax.ShapeDtypeStruct(
                (NCORES * shp[0],) + shp[1:], mybir.dt.np(alloc.dtype),
                sharding=sh)
    sds = [gshape[nm] for nm in all_in_names if nm != partition_name]
    try:
        compiled = bass2jax.fast_dispatch_compile(
            lambda: fn.lower(*sds).compile())
    except Exception:
        compiled = fn

    cpu, quant_x, dequant_out = _host_fns()

    # out staging buffers: uploaded once, never donated, so the NEFF's
    # result buffers are freshly allocated each call (kernel writes
    # every output element; no zero-init needed)
    zeros = jax.device_put(
        np.zeros((NCORES * BL, N, OROW), np.uint8), sh)

    st = dict(nc=nc, fn=compiled, sh=sh, cpu=cpu, quant_x=quant_x,
              dequant_out=dequant_out, in_names=in_names, zeros=zeros,
              devices=devices, weights=None, dev_w=None)
    _CACHE["st"] = st
    return st


def _upload_weights(st, W_qkv, W_proj, b_proj):
    import jax

    def same(a, b):
        return a is b or np.array_equal(a, b)

    w = st["weights"]
    if (w is not None and same(w[0], W_qkv) and same(w[1], W_proj)
            and same(w[2], b_proj)):
        return
    mfull = _local_mask_T()
    maskT = np.concatenate(
        [mfull[128 * j:128 * (j + 1), _mask_base(j):_mask_base(j) + MB]
         for j in range(NT)], axis=0).astype(ml_dtypes.bfloat16)
    wqkv = W_qkv.astype(ml_dtypes.bfloat16)
    wproj = W_proj.astype(ml_dtypes.bfloat16)
    bproj = np.ascontiguousarray(b_proj.reshape(1, C)).astype(
        ml_dtypes.bfloat16)
    eye = np.eye(128, dtype=np.float32).astype(ml_dtypes.bfloat16)
    dev = {
        "wqkv": np.concatenate([wqkv] * NCORES, axis=0),
        "wproj": np.concatenate([wproj] * NCORES, axis=0),
        "bproj": np.concatenate([bproj] * NCORES, axis=0),
        "maskT": np.concatenate([maskT] * NCORES, axis=0),
        "eye": np.concatenate([eye] * NCORES, axis=0),
    }
    st["dev_w"] = {k: jax.device_put(v, st["sh"]) for k, v in dev.items()}
    st["weights"] = (W_qkv.copy(), W_proj.copy(), b_proj.copy())


def kernel(x, W_qkv, W_proj, b_proj):
    import jax

    x = np.asarray(x, dtype=np.float32)
    W_qkv = np.asarray(W_qkv, dtype=np.float32)
    W_proj = np.asarray(W_proj, dtype=np.float32)
    b_proj = np.asarray(b_proj, dtype=np.float32)

    st = _state()

    # per-core chunk quantize + shard upload: the wire transfer of shard
    # c overlaps the CPU quantization of shard c+1
    shards = []
    with jax.default_device(st["cpu"]):
        for c in range(NCORES):
            qc = st["quant_x"](x[c * BL:(c + 1) * BL])
            qc.block_until_ready()
            shards.append(jax.device_put(qc, st["devices"][c]))
    xd = jax.make_array_from_single_device_arrays(
        (B, N, OROW), st["sh"], shards)

    # weight check/upload after the x puts are in flight: the compare
    # (or the rare re-upload) overlaps the x wire transfer
    _upload_weights(st, W_qkv, W_proj, b_proj)

    args = []
    for nm in st["in_names"]:
        args.append(xd if nm == "xq" else st["dev_w"][nm])
    args.append(st["zeros"])
    outs = st["fn"](*args)
    return st["dequant_out"](outs[0])
